# revision 1
# baseline (speedup 1.0000x reference)
"""Trainium2 Bass kernel for nn_EnergyBalanceChecker (segment_reduce).

Problem (hardcoded): B=4, N=512, T=24, G=32, TOL=0.05, EPS=1e-6.

  onehot[g,n] = (lv_group_ids[n] == g);  M = onehot * valid_lv_mask
  gc  = einsum('gn,bnt->bgt', M, consumption)
  gg  = einsum('gn,bnt->bgt', M, generation)
  net = einsum('gn,bnt->bgt', M, S.sum(axis=2) - S.sum(axis=1))
  pen = relu(|gc-gg+net| / (gc+gg+eps) - TOL);  out = pen.sum()*w/n_unique

Sharding: 8 cores = 4 batches x 2 halves of the (row) N axis. Each core
reads the contiguous block S[b, h*256:(h+1)*256, :, :] (12.6 MB) once
(SWDGE-cast to fp16 in flight) and emits per-group partials [3, 32, 24]
= (pgc, pgg, pnet). A single PE pass computes both reductions at once:
lhsT = [M^T_loc | ones] gives PSUM rows 0..31 = M-projected rows (still
per-(m,t)) and row 32 = plain column sums. Both PSUM readers run
concurrently: DVE reduces rows 0..31 over m directly from PSUM (row-sum
term) while ACT keeps row 32, which is regathered to m-partitions per
64 columns and folded in with -M^T_full weights (deferred matmuls).
Host sums the two half partials per batch and applies the tiny
nonlinear tail (~20 KFLOP).
"""

import sys

import numpy as np

try:
    import concourse  # noqa: F401
except ImportError:
    sys.path.insert(0, "/opt/trn_rl_repo")

import concourse.tile as tile
from concourse import bacc, mybir
from concourse.bass_utils import run_bass_kernel_spmd

B, N, T, G = 4, 512, 24, 32
TOL, EPS = 0.05, 1e-6
P = 128                 # SBUF partitions
NLOC = N // 2           # rows per core (n-half)
NB = NLOC // P          # 2 n-blocks of 128 rows
QM = 64                 # m-columns per streamed S tile
MB = N // QM            # number of (nb-pair) S tiles per core
FREE = QM * T           # free elements per (nb, mb) tile
MMCH = 512              # matmul free-dim chunk
EV = 768                # PSUM evacuation chunk (2 banks, 32 m-columns)
NEV = N * T // EV       # total evacuation chunks (16)
EVM = EV // T           # m-columns per evacuation chunk (32)
CT = N // P             # colT regather chunks of 128 m (4)

_F32 = mybir.dt.float32
_F16 = mybir.dt.float16


def _build_program():
    nc = bacc.Bacc("TRN2", target_bir_lowering=False, debug=False,
                   enable_asserts=False, num_devices=8)
    s = nc.dram_tensor("s", [NLOC, N, T], _F32, kind="ExternalInput").ap()
    cons = nc.dram_tensor("cons", [NLOC, T], _F32, kind="ExternalInput").ap()
    gen = nc.dram_tensor("gen", [NLOC, T], _F32, kind="ExternalInput").ap()
    mt_loc = nc.dram_tensor("mt_loc", [NLOC, G], _F32, kind="ExternalInput").ap()
    mt_neg = nc.dram_tensor("mt_neg", [N, G], _F32, kind="ExternalInput").ap()
    out = nc.dram_tensor("out", [3, G, T], _F32, kind="ExternalOutput").ap()

    with tile.TileContext(nc) as tc:
        with (
            tc.tile_pool(name="spool", bufs=NB * MB) as spool,
            tc.tile_pool(name="small", bufs=1) as small,
            tc.tile_pool(name="pcol", bufs=3, space="PSUM") as pcol,
            tc.tile_pool(name="pproj", bufs=1, space="PSUM") as pproj,
        ):
            # lhsT for the main pass: columns 0..31 = M^T rows for this
            # n-block, column 32 = ones (plain column sum). fp16, like the
            # streamed S tiles, for full-rate PE; PSUM accumulates fp32.
            lhsT32 = small.tile([P, NB, G], _F32, tag="lhsT32")
            nc.scalar.dma_start(out=lhsT32,
                                in_=mt_loc.rearrange("(nb p) g -> p nb g", p=P))
            # Cast on DVE (32 cycles) rather than a SWDGE cast-DMA: the
            # Q7's descriptor emission must stay free for the S stream.
            lhsT = small.tile([P, NB, G + 1], _F16, tag="lhsT")
            nc.vector.tensor_copy(out=lhsT[:, :, 0:G], in_=lhsT32)
            nc.vector.memset(lhsT[:, :, G:G + 1], 1.0)
            mtn = small.tile([P, CT, G], _F32, tag="mtn")
            nc.scalar.dma_start(out=mtn, in_=mt_neg.rearrange("(mc p) g -> p mc g", p=P))
            cg = small.tile([P, 2, NB, T], _F32, tag="cg")
            nc.scalar.dma_start(out=cg[:, 0], in_=cons.rearrange("(nb p) t -> p nb t", p=P))
            nc.scalar.dma_start(out=cg[:, 1], in_=gen.rearrange("(nb p) t -> p nb t", p=P))

            colacc = small.tile([1, N * T], _F32, tag="colacc")
            colT = small.tile([P, CT, T], _F32, tag="colT")
            rowacc = small.tile([G, NEV, T], _F32, tag="rowacc")
            rowsum = small.tile([G, T], _F32, tag="rowsum")
            out_sb = small.tile([G, 3, T], _F32, tag="out_sb")

            pgc = pproj.tile([G, T], _F32, tag="pgc")
            pgg = pproj.tile([G, T], _F32, tag="pgg")
            pcp = pproj.tile([G, T], _F32, tag="pgc")  # reuses pgc's bank (pgc retires early)

            # pgc / pgg: tiny projections of consumption / generation.
            for nb in range(NB):
                nc.tensor.matmul(pgc, lhsT32[:, nb], cg[:, 0, nb],
                                 start=(nb == 0), stop=(nb == NB - 1))
                nc.tensor.matmul(pgg, lhsT32[:, nb], cg[:, 1, nb],
                                 start=(nb == 0), stop=(nb == NB - 1))
            nc.scalar.copy(out=out_sb[:, 0], in_=pgc)
            nc.scalar.copy(out=out_sb[:, 1], in_=pgg)
            nc.scalar.dma_start(out=out[0:2].rearrange("k g t -> g k t"),
                                in_=out_sb[:, 0:2])

            # Stream all of S up front on the SP HWDGE ring.
            s4 = s.rearrange("(nb p) (mb q) t -> mb nb p (q t)", p=P, q=QM)
            stiles = {}
            for mb in range(MB):
                for nb in range(NB):
                    st = spool.tile([P, FREE], _F16, tag="s")
                    nc.gpsimd.dma_start(out=st, in_=s4[mb, nb])
                    stiles[(mb, nb)] = st

            # Flat loop over the 16 evacuation chunks (32 m-columns each).
            # nb outer within each PSUM tile so the stationary weights
            # reload NB times per tile, not per matmul.
            for q in range(NEV):
                pos = q * EV                    # global (m, t) flat offset
                mb, off = divmod(pos, FREE)     # source S tile and offset
                pc = pcol.tile([G + 1, EV], _F32, tag="pc")
                for nb in range(NB):
                    for c0 in range(0, EV, MMCH):
                        cw = min(MMCH, EV - c0)
                        nc.tensor.matmul(
                            pc[:, c0:c0 + cw],
                            lhsT[:, nb],
                            stiles[(mb, nb)][:, off + c0:off + c0 + cw],
                            start=(nb == 0), stop=(nb == NB - 1),
                            skip_group_check=True)
                # Two independent readers drain PSUM concurrently: ACT
                # keeps only the column-sum row, DVE folds the projected
                # rows over this chunk's 32 m-columns.
                nc.scalar.copy(out=colacc[:, pos:pos + EV],
                               in_=pc[G:G + 1, :])
                nc.vector.reduce_sum(
                    out=rowacc[:, q],
                    in_=pc[0:G, :].rearrange("p (m t) -> p t m", t=T),
                    axis=mybir.AxisListType.X,
                )
                # At each tile (64-m) boundary: column sums to
                # m-partitions (ACT HWDGE ring, tiny), so the last
                # regather only waits on the final evacuation.
                if (q + 1) % (QM // EVM) == 0:
                    ct = q // (QM // EVM)
                    po = QM * (ct % (P // QM))
                    nc.sync.dma_start(
                        out=colT[po:po + QM, ct // (P // QM), :],
                        in_=colacc[0:1, ct * QM * T:(ct + 1) * QM * T].rearrange(
                            "p (m t) -> p m t", t=T))

            # Deferred -M^T @ colsum matmuls (K=64 each; PE is in-order,
            # inlining them would stall the chunk stream on colT DMAs).
            for ct in range(MB):
                po = QM * (ct % (P // QM))
                nc.tensor.matmul(pcp, mtn[po:po + QM, ct // (P // QM), :],
                                 colT[po:po + QM, ct // (P // QM), :],
                                 start=(ct == 0), stop=(ct == MB - 1),
                                 skip_group_check=True)

            nc.vector.reduce_sum(
                out=rowsum, in_=rowacc[:].rearrange("p e t -> p t e"),
                axis=mybir.AxisListType.X)
            nc.vector.tensor_add(out_sb[:, 2], rowsum, pcp)
            nc.sync.dma_start(out=out[2], in_=out_sb[:, 2])
    nc.compile()
    # Drop the framework's const-tensor memsets (const-float32-0.0 etc.):
    # nothing in this program reads them (birverifier confirms), but they
    # run on the Pool engine ahead of the barrier and delay the first
    # SWDGE descriptor emission of the S stream by ~0.4 us.
    for blk in nc.m.functions[0].blocks:
        blk.instructions = [
            i for i in blk.instructions
            if not (type(i).__name__ == "InstMemset"
                    and i.outs and "const-" in str(i.outs[0]))
        ]
    return nc


_NC_CACHE = None


def _get_program():
    global _NC_CACHE
    if _NC_CACHE is None:
        _NC_CACHE = _build_program()
    return _NC_CACHE


_RUNNER_CACHE = None


def _get_runner():
    """Compiled-once jit(shard_map) executor over 8 cores.

    Mirrors concourse.bass2jax.run_bass_via_pjrt but caches the traced
    function so repeat calls skip retracing/compile-cache lookups."""
    global _RUNNER_CACHE
    if _RUNNER_CACHE is None:
        import jax
        from jax.sharding import Mesh, PartitionSpec
        from jax.experimental.shard_map import shard_map
        from concourse import bass2jax, mybir as mb

        nc = _get_program()
        bass2jax.install_neuronx_cc_hook()
        partition_name = (nc.partition_id_tensor.name
                          if nc.partition_id_tensor else None)
        in_names, out_names, out_avals = [], [], []
        for alloc in nc.m.functions[0].allocations:
            if not isinstance(alloc, mb.MemoryLocationSet):
                continue
            name = alloc.memorylocations[0].name
            if alloc.kind == "ExternalInput":
                if name != partition_name:
                    in_names.append(name)
            elif alloc.kind == "ExternalOutput":
                out_names.append(name)
                out_avals.append(jax.core.ShapedArray(
                    tuple(alloc.tensor_shape), mb.dt.np(alloc.dtype)))
        n_params = len(in_names)
        all_names = in_names + out_names
        if partition_name is not None:
            all_names = all_names + [partition_name]

        def _body(*args):
            operands = list(args)
            if partition_name is not None:
                operands.append(bass2jax.partition_id_tensor())
            outs = bass2jax._bass_exec_p.bind(
                *operands,
                out_avals=tuple(out_avals),
                in_names=tuple(all_names),
                out_names=tuple(out_names),
                lowering_input_output_aliases=(),
                sim_require_finite=True,
                sim_require_nnan=True,
                nc=nc,
            )
            return tuple(outs)

        devices = jax.devices()[:8]
        mesh = Mesh(np.asarray(devices), ("core",))
        n_outs = len(out_names)
        sharded = jax.jit(
            shard_map(_body, mesh=mesh,
                      in_specs=(PartitionSpec("core"),) * (n_params + n_outs),
                      out_specs=(PartitionSpec("core"),) * n_outs,
                      check_rep=False),
            donate_argnums=tuple(range(n_params, n_params + n_outs)),
            keep_unused=True,
        )
        _RUNNER_CACHE = (sharded, in_names[:n_params], out_names, out_avals)
    return _RUNNER_CACHE


def kernel(consumption, generation, sharing_matrix, lv_group_ids,
           valid_lv_mask, imbalance_penalty_weight, _want_results=False,
           **run_kwargs):
    consumption = np.ascontiguousarray(consumption, dtype=np.float32)
    generation = np.ascontiguousarray(generation, dtype=np.float32)
    sharing_matrix = np.ascontiguousarray(sharing_matrix, dtype=np.float32)
    ids = np.asarray(lv_group_ids)
    valid = np.asarray(valid_lv_mask, dtype=np.float32)
    w = np.float32(np.asarray(imbalance_penalty_weight))

    onehot = (ids[None, :] == np.arange(G)[:, None]).astype(np.float32)
    n_unique = np.float32(np.unique(ids).size)
    M = onehot * valid[None, :]                      # [G, N]
    mt = np.ascontiguousarray(M.T)                   # [N, G]
    mt_neg = np.ascontiguousarray(-mt)

    in_maps = []
    for c in range(8):
        b, h = divmod(c, 2)
        sl = slice(h * NLOC, (h + 1) * NLOC)
        in_maps.append({
            "s": np.ascontiguousarray(sharing_matrix[b, sl]),
            "cons": np.ascontiguousarray(consumption[b, sl]),
            "gen": np.ascontiguousarray(generation[b, sl]),
            "mt_loc": np.ascontiguousarray(mt[sl]),
            "mt_neg": mt_neg,
        })
    res = None
    if _want_results or run_kwargs:
        nc = _get_program()
        res = run_bass_kernel_spmd(nc, in_maps, core_ids=list(range(8)),
                                   **run_kwargs)
        parts = np.stack([res.results[c]["out"] for c in range(8)])
    else:
        try:
            fn, in_names, out_names, out_avals = _get_runner()
            concat_in = [np.concatenate([m[name] for m in in_maps], axis=0)
                         for name in in_names]
            zeros = [np.zeros((8 * a.shape[0], *a.shape[1:]), a.dtype)
                     for a in out_avals]
            out_arrs = fn(*concat_in, *zeros)
            parts = np.asarray(out_arrs[out_names.index("out")]).reshape(
                8, 3, G, T)
        except Exception:
            nc = _get_program()
            res = run_bass_kernel_spmd(nc, in_maps, core_ids=list(range(8)))
            parts = np.stack([res.results[c]["out"] for c in range(8)])
    full = parts.reshape(B, 2, 3, G, T).sum(axis=1, dtype=np.float32)
    gc, gg, net = full[:, 0], full[:, 1], full[:, 2]

    imbalance = np.abs(gc - gg + net)
    total = gc + gg + np.float32(EPS)
    pen = np.maximum(imbalance / total - np.float32(TOL), np.float32(0))
    outv = np.float32(pen.sum(dtype=np.float32) * w / n_unique)
    out_arr = np.array(outv, dtype=np.float32)
    if _want_results:
        return out_arr, res
    return out_arr



# revision 3
# speedup vs baseline: 1.0191x; 1.0191x over previous
"""Trainium2 Bass kernel for nn_EnergyBalanceChecker (segment_reduce), v4.

Problem (hardcoded): B=4, N=512, T=24, G=32, TOL=0.05, EPS=1e-6.

  M = onehot(lv_group_ids) * valid_lv_mask                     # [G, N]
  gc  = einsum('gn,bnt->bgt', M, consumption)
  gg  = einsum('gn,bnt->bgt', M, generation)
  net = einsum('gn,bnt->bgt', M, S.sum(axis=2) - S.sum(axis=1))
  pen = relu(|gc-gg+net| / (gc+gg+eps) - TOL);  out = pen.sum()*w/n_unique

Sharding: 8 cores = 4 batches x 2 halves of the (row) N axis.

Dataflow:
  * S streams in fp8e4 (SWDGE cast in flight): DMA cost is charged on
    destination bytes, so this halves the 17.5us fp16 stream floor.
  * The 128 SBUF partitions carry (na in 8 n-rows) x (msub in 16 m-blocks),
    so each matmul contracts 16 m-positions along with n. 32 blocks of 8
    n-rows cover the shard; lhsT = [M-projection rows | 16 msub indicator
    rows] (indicator rows preserve per-m column sums).
  * Blocks pair into fp8 DoubleRow matmuls (2 k-tiles/pass, 0.5 cyc/col)
    accumulating into per-half PSUM regions, column-split into [*,512] +
    [*,256] tiles so the two drain copies (ACT / DVE) run concurrently.
  * Drain: copy region to f16, then PE folds the q axis -- identity-matmul
    the projection rows (+row term) and (-M-slice)-matmul the colsum rows
    (-col term) -- accumulating net = row - col directly into one PSUM tile.
  * Host does only the [3, G, T]-level nonlinear tail.
"""

import sys

import numpy as np

try:
    import concourse  # noqa: F401
except ImportError:
    sys.path.insert(0, "/opt/trn_rl_repo")

import ml_dtypes

import concourse.tile as tile
from concourse import bacc, mybir
from concourse.bass_utils import run_bass_kernel_spmd

B, N, T, G = 4, 512, 24, 32
TOL, EPS = 0.05, 1e-6
P = 128                 # SBUF partitions
NLOC = N // 2           # rows per core (n-half)
A = 8                   # n-rows per block (partition sub-dim)
C = 16                  # m-blocks on partitions (partition sub-dim)
Q = N // C              # m-columns per msub block (free dim)
NBLK = NLOC // A        # 32 blocks of 8 n-rows
PAIRS = NBLK // 2       # DoubleRow pairs
F = Q * T               # free elements per block / PSUM region columns
GC = G + C              # lhsT columns: G projection + C indicator rows
DMA_BLOCKS = ((0, 5), (5, 11), (11, 17), (17, 23), (23, 28),
              (28, 30), (30, 32))   # stream DMA block ranges
CHUNKS = ((0, 512), (512, F - 512))   # PSUM bank-aligned matmul chunks
OROW = 128              # padded f32 row per group in the scatter-add out

_F32 = mybir.dt.float32
_F16 = mybir.dt.float16
_F8 = mybir.dt.float8e4
_U8 = mybir.dt.uint8
_I16 = mybir.dt.int16


def _build_program():
    nc = bacc.Bacc("TRN2", target_bir_lowering=False, debug=False,
                   enable_asserts=False, num_devices=8)
    s = nc.dram_tensor("s", [NLOC, N, T], _F32, kind="ExternalInput").ap()
    sm_in = nc.dram_tensor("sm_in", [P, 2 * (G + 2 * T)], _F32,
                           kind="ExternalInput").ap()
    lhs8 = nc.dram_tensor("lhs8", [P, NBLK, GC], _U8, kind="ExternalInput").ap()
    msl = nc.dram_tensor("msl", [C, Q, G], _F16, kind="ExternalInput").ap()
    out = nc.dram_tensor("out", [3, G, T], _F32, kind="ExternalOutput").ap()

    with tile.TileContext(nc) as tc:
        with (
            tc.tile_pool(name="sb", bufs=1) as sb,
            tc.tile_pool(name="ps", bufs=1, space="PSUM") as ps,
        ):
            # --- small inputs on the SP HWDGE ring ---
            ltile = sb.tile([P, NBLK, GC], _U8, tag="ltile")
            nc.sync.dma_start(out=ltile, in_=lhs8)
            # msl lands on partitions G..G+C so the col-matmul lhsT shares the
            # base partition of the colsum rows in pcopy.
            msl_sb = sb.tile([GC, Q, G], _F16, tag="msl")
            nc.sync.dma_start(out=msl_sb[G:GC], in_=msl)
            lhsT = ltile.bitcast(_F8)

            stile = sb.tile([P, NBLK, F], _F8, tag="stile")
            pcopy = sb.tile([GC, 2, F], _F16, tag="pcopy")
            small = sb.tile([P, 2, G + 2 * T], _F32, tag="small")
            rs = sb.tile([G, 2, T], _F32, tag="rs")
            rsum = sb.tile([G, T], _F32, tag="rsum")
            osb = sb.tile([G, 3, T], _F32, tag="osb")

            # Per-half regions, column-split so the two drain copies hit
            # different PSUM tiles and run concurrently (the dep tracker
            # serializes readers of a single PSUM tile).
            reg = [(ps.tile([GC, CHUNKS[0][1]], _F32, name=f"reg{h}a"),
                    ps.tile([GC, CHUNKS[1][1]], _F32, name=f"reg{h}b"))
                   for h in range(2)]
            bt = ps.tile([G, T], _F32, tag="bt")
            pg = ps.tile([G, 2 * T], _F32, tag="pg")

            # --- S stream: fp8 cast DMAs on the SWDGE ring ---
            # partition p = na*C + msub; block blk: n = blk*A + na;
            # free = (mq, t) with m = msub*Q + mq.
            s_r = s.rearrange("(blk a) (c q) t -> (a c) blk (q t)", a=A, c=C)
            for b0, b1 in DMA_BLOCKS:
                nc.gpsimd.dma_start(
                    out=stile[:, b0:b1, :].rearrange("p b f -> p (b f)"),
                    in_=s_r[:, b0:b1, :])
            # sm_in rides the SWDGE ring after the stream so its transfer
            # lands past the last S byte instead of injecting mid-stream.
            nc.gpsimd.dma_start(
                out=small.rearrange("p nb f -> p (nb f)"), in_=sm_in)

            # --- main pass: DoubleRow matmuls, two half-regions ---
            # Drain: ACT copies cols 0:512 and DVE cols 512:768 to f16
            # concurrently (different tiles -> no reader serialization); then
            # DVE reduces the projection rows over q while PE's -M col-matmuls
            # fold the colsum rows into bt.
            def drain(h):
                nc.scalar.copy(out=pcopy[:, h, 0:CHUNKS[0][1]], in_=reg[h][0])
                nc.vector.tensor_copy(out=pcopy[:, h, CHUNKS[0][1]:],
                                      in_=reg[h][1])
                nc.vector.reduce_sum(
                    out=rs[:, h],
                    in_=pcopy[0:G, h, :].rearrange("p (q t) -> p t q", t=T),
                    axis=mybir.AxisListType.X)
                for q in range(Q):
                    nc.tensor.matmul(
                        bt, msl_sb[G:GC, q],
                        pcopy[G:GC, h, q * T:(q + 1) * T],
                        start=(h == 0 and q == 0),
                        stop=(h == 1 and q == Q - 1),
                        skip_group_check=True)

            for h in range(2):
                for j in range(PAIRS // 2):
                    pair = h * (PAIRS // 2) + j
                    for k, (c0, cw) in enumerate(CHUNKS):
                        nc.tensor.matmul(
                            reg[h][k],
                            lhsT[:, 2 * pair:2 * pair + 2, :],
                            stile[:, 2 * pair:2 * pair + 2, c0:c0 + cw],
                            start=(j == 0), stop=(j == PAIRS // 2 - 1),
                            perf_mode=mybir.MatmulPerfMode.DoubleRow,
                            skip_group_check=True)
                if h == 1:
                    # gc|gg projections in one region (cons/gen columns are
                    # adjacent in small): sm_in lands right after the stream,
                    # so these run in the post-stream PE window.
                    lhsT32 = small[:, :, 0:G]
                    for nb in range(2):
                        nc.tensor.matmul(pg, lhsT32[:, nb],
                                         small[:, nb, G:],
                                         start=(nb == 0), stop=(nb == 1))
                drain(h)

            # --- merge row + col terms, single out DMA ---
            nc.scalar.copy(out=osb[:, 0:2].rearrange("g k t -> g (k t)"),
                           in_=pg)
            nc.vector.tensor_add(rsum, rs[:, 0], rs[:, 1])
            nc.vector.tensor_add(osb[:, 2], rsum, bt)
            nc.sync.dma_start(out=out.rearrange("k g t -> g k t"), in_=osb)
    nc.compile()
    # Drop the framework's const-tensor memsets: nothing reads them, but they
    # run on the Pool engine ahead of the barrier and delay the first SWDGE
    # descriptor emission of the S stream.
    for blk in nc.m.functions[0].blocks:
        blk.instructions = [
            i for i in blk.instructions
            if not (type(i).__name__ == "InstMemset"
                    and i.outs and "const-" in str(i.outs[0]))
        ]
    return nc


_NC_CACHE = None


def _get_program():
    global _NC_CACHE
    if _NC_CACHE is None:
        _NC_CACHE = _build_program()
    return _NC_CACHE


_RUNNER_CACHE = None


def _get_runner():
    """Compiled-once jit(shard_map) executor over 8 cores."""
    global _RUNNER_CACHE
    if _RUNNER_CACHE is None:
        import jax
        from jax.sharding import Mesh, PartitionSpec
        from jax.experimental.shard_map import shard_map
        from concourse import bass2jax, mybir as mb

        nc = _get_program()
        bass2jax.install_neuronx_cc_hook()
        partition_name = (nc.partition_id_tensor.name
                          if nc.partition_id_tensor else None)
        in_names, out_names, out_avals = [], [], []
        for alloc in nc.m.functions[0].allocations:
            if not isinstance(alloc, mb.MemoryLocationSet):
                continue
            name = alloc.memorylocations[0].name
            if alloc.kind == "ExternalInput":
                if name != partition_name:
                    in_names.append(name)
            elif alloc.kind == "ExternalOutput":
                out_names.append(name)
                out_avals.append(jax.core.ShapedArray(
                    tuple(alloc.tensor_shape), mb.dt.np(alloc.dtype)))
        n_params = len(in_names)
        all_names = in_names + out_names
        if partition_name is not None:
            all_names = all_names + [partition_name]

        def _body(*args):
            operands = list(args)
            if partition_name is not None:
                operands.append(bass2jax.partition_id_tensor())
            outs = bass2jax._bass_exec_p.bind(
                *operands,
                out_avals=tuple(out_avals),
                in_names=tuple(all_names),
                out_names=tuple(out_names),
                lowering_input_output_aliases=(),
                sim_require_finite=True,
                sim_require_nnan=True,
                nc=nc,
            )
            return tuple(outs)

        devices = jax.devices()[:8]
        mesh = Mesh(np.asarray(devices), ("core",))
        n_outs = len(out_names)
        sharded = jax.jit(
            shard_map(_body, mesh=mesh,
                      in_specs=(PartitionSpec("core"),) * (n_params + n_outs),
                      out_specs=(PartitionSpec("core"),) * n_outs,
                      check_rep=False),
            donate_argnums=tuple(range(n_params, n_params + n_outs)),
            keep_unused=True,
        )
        _RUNNER_CACHE = (sharded, in_names[:n_params], out_names, out_avals)
    return _RUNNER_CACHE


def _host_side(consumption, generation, sharing_matrix, lv_group_ids,
               valid_lv_mask):
    """Shared input prep: per-core input maps."""
    consumption = np.ascontiguousarray(consumption, dtype=np.float32)
    generation = np.ascontiguousarray(generation, dtype=np.float32)
    sharing_matrix = np.ascontiguousarray(sharing_matrix, dtype=np.float32)
    ids = np.asarray(lv_group_ids)
    valid = np.asarray(valid_lv_mask, dtype=np.float32)

    onehot = (ids[None, :] == np.arange(G)[:, None]).astype(np.float32)
    n_unique = np.float32(np.unique(ids).size)
    M = onehot * valid[None, :]                      # [G, N]
    mt = np.ascontiguousarray(M.T)                   # [N, G]

    # msl[msub, q, g] = -M[g, msub*Q + q]  (negative col weights baked in)
    msl = np.ascontiguousarray((-mt).reshape(C, Q, G).astype(np.float16))

    in_maps = []
    for c in range(8):
        b, hh = divmod(c, 2)
        sl = slice(hh * NLOC, (hh + 1) * NLOC)
        mt_half = mt[sl]                             # [NLOC, G]
        # lhs8[p=(na,msub), blk, :G] = M[g, blk*A + na]; [:, :, G+j] = (msub==j)
        proj = mt_half.reshape(NBLK, A, G).transpose(1, 0, 2)   # [na, blk, g]
        proj = np.broadcast_to(proj[:, None], (A, C, NBLK, G))
        ind = np.broadcast_to(
            np.tile(np.eye(C, dtype=np.float32), (A, 1))[:, None, :],
            (P, NBLK, C))
        lhs = np.concatenate(
            [proj.reshape(P, NBLK, G), ind], axis=2)            # [P, NBLK, GC]
        lhs8 = np.ascontiguousarray(
            lhs.astype(ml_dtypes.float8_e4m3).view(np.uint8))
        # sm_in[p, (nb, f)]: f = [mt row | cons row | gen row] for n = nb*P + p
        sm = np.empty((2, P, G + 2 * T), np.float32)
        sm[:, :, :G] = mt_half.reshape(2, P, G)
        sm[:, :, G:G + T] = consumption[b, sl].reshape(2, P, T)
        sm[:, :, G + T:] = generation[b, sl].reshape(2, P, T)
        in_maps.append({
            "s": np.ascontiguousarray(sharing_matrix[b, sl]),
            "sm_in": np.ascontiguousarray(
                sm.transpose(1, 0, 2).reshape(P, -1)),
            "lhs8": lhs8,
            "msl": msl,
        })
    return in_maps, n_unique


def kernel(consumption, generation, sharing_matrix, lv_group_ids,
           valid_lv_mask, imbalance_penalty_weight, _want_results=False,
           **run_kwargs):
    w = np.float32(np.asarray(imbalance_penalty_weight))
    in_maps, n_unique = _host_side(consumption, generation, sharing_matrix,
                                   lv_group_ids, valid_lv_mask)
    res = None
    if _want_results or run_kwargs:
        nc = _get_program()
        res = run_bass_kernel_spmd(nc, in_maps, core_ids=list(range(8)),
                                   **run_kwargs)
        parts = np.stack([res.results[c]["out"] for c in range(8)])
    else:
        try:
            fn, in_names, out_names, out_avals = _get_runner()
            concat_in = [np.concatenate([m[name] for m in in_maps], axis=0)
                         for name in in_names]
            zeros = [np.zeros((8 * a.shape[0], *a.shape[1:]), a.dtype)
                     for a in out_avals]
            out_arrs = fn(*concat_in, *zeros)
            parts = np.asarray(out_arrs[out_names.index("out")]).reshape(
                8, 3, G, T)
        except Exception:
            nc = _get_program()
            res = run_bass_kernel_spmd(nc, in_maps, core_ids=list(range(8)))
            parts = np.stack([res.results[c]["out"] for c in range(8)])
    full = parts.reshape(B, 2, 3, G, T).sum(axis=1, dtype=np.float32)
    gc, gg, net = full[:, 0], full[:, 1], full[:, 2]

    imbalance = np.abs(gc - gg + net)
    total = gc + gg + np.float32(EPS)
    pen = np.maximum(imbalance / total - np.float32(TOL), np.float32(0))
    outv = np.float32(pen.sum(dtype=np.float32) * w / n_unique)
    out_arr = np.array(outv, dtype=np.float32)
    if _want_results:
        return out_arr, res
    return out_arr
